# revision 60
# baseline (speedup 1.0000x reference)
"""Trainium2 SPMD kernel for nn_AutoregressiveDecoder (gnn_message_passing).

Math (reference, per context g in 0..N-1, N=384):
    h1[g]  = concat(z, e_g) @ W1                        # = H0 + e_g (x) W1r
    A[g]   = relu(P_g @ h1[g])         P_g = partials[g]
    h2[g]  = A[g] @ W2
    h3[g]  = P_g @ h2[g]
    S[g,:] = h3[g][g,:] @ h3[g].T      (row g of supplement, pre-tril)
    out    = x + 0.5*(tril(S) + tril(S).T)

8 cores x 48 contexts, interleaved assignment g = 8b + c (slot b on core c)
so the tril truncation width 8b+8 >= g+1 is static in the shared program.
Per slot b (software-pipelined, skew 3):
    mm1  A_T[h,:]  = sum_j H0m[j,h] Pt[j,:]: contraction chunks 0,1 as ONE
         fp8e4 DoubleRow matmul (2 k-tiles contracted per column pass, so
         half the column count of two bf16 matmuls), chunk 2 in bf16.
         The rank-1 e_g (x) W1r update is folded into a per-slot copy of
         H0's chunk b//16 (row g += W1r) -- no K=1 matmuls.
    mm2  h2[j,k]   = sum_h A_T[h,j] W2[h,k]   bf16 F=128 x6
    mm3  h3T[k,:]  = sum_j h2[j,k] PtAug[j,:] bf16 F=8b+10 x3, moving col 0
         is prow so h3T[:,0] = d = h3[g][g,:]  (separate bf16 copy of the
         truncated Pt columns; the fp8 copy is only used by mm1)
    mm4  S[1,:]    = sum_k d[k] h3T[k,1:]     bf16 F=8b+8 (psum row aliased)
tril/symmetrize/(+x) happen on host at unshard.
PE stream at iter i: mm1(i), mm2(i-1), mm3(i-2), mm4(i-3); ACT does the
relus, DVE does S-row drain + h2/h3 PSUM->SBUF copies (single ordered
semaphore so the PE needs only ~3 waits per iteration).
"""

import os
from contextlib import ExitStack

import numpy as np
import ml_dtypes

import concourse.bass as bass
import concourse.mybir as mybir
from concourse.bass_utils import run_bass_kernel_spmd

N = 384
D = 128
HID = 256
HID2 = 128
NCORES = 8
NB = N // NCORES  # 48 contexts per core
W = N + 2  # ptB width: col 0 = prow, cols 1..384 = Pt, col 385 pad
FWMAX = 8 * 47 + 10  # widest mm3 moving slice (slot 47)
PTBUF = 8  # pt SBUF ring depth
SRBUF = 8  # S-row SBUF ring depth

F32 = mybir.dt.float32
BF16 = mybir.dt.bfloat16
FP8 = mybir.dt.float8e4
DR = mybir.MatmulPerfMode.DoubleRow
AFT = mybir.ActivationFunctionType
BF = ml_dtypes.bfloat16
F8 = ml_dtypes.float8_e4m3fn

_NC_CACHE = {}
LAST_RESULT = None  # test.py reads exec_time_ns from here


def _jw(b):
    """Truncated S-row width for slot b: covers j <= g for all g = 8b+c."""
    return 8 * b + 8


def _fw(b):
    """mm3 moving width: prow col + j cols + pad to even."""
    return _jw(b) + 2


def _h0m_piece(b):
    """h0m DMA piece covering slot b (piece 0 = slot 0 alone so the first
    mm1 is gated on a tiny DMA). Pieces 0-4 live in h0mA (fp8 DR pairs,
    slots 0-31), pieces 5-6 in h0mB (bf16 chunk-2 copies, slots 32-47)."""
    return 0 if b == 0 else 1 if b < 8 else 1 + b // 8


N_H0M_PIECES = 7


def _build_nc() -> bass.Bass:
    nc = bass.Bass()
    # pts: ONE bf16 blob per slot (single DMA -- descriptor generation for
    # multiple DMAs per slot is the bottleneck, ~1us each on a queue):
    #   [0, N)              fp8 ptA bytes bitcast as bf16: DR moving pair,
    #                       chunks 0,1, [p, i, j] flattened i*N+j
    #   [N, N+W)            chunk-2 tile (prow col 0, Pt cols 1..384, pad)
    #   [N+W + t*fw, ...)   truncated chunk-t tile for mm3 (t in 0,1)
    btot = sum(N + W + 2 * _fw(b) for b in range(NB))
    pts_d = nc.declare_dram_parameter("pts", [128, btot], BF16, isOutput=False)
    h0fA_d = nc.declare_dram_parameter("h0fA", [128, 2, HID], FP8, isOutput=False)
    h0fB_d = nc.declare_dram_parameter("h0fB", [128, HID], BF16, isOutput=False)
    h0mA_d = nc.declare_dram_parameter("h0mA", [128, 2, 32 * HID], FP8, isOutput=False)
    h0mB_d = nc.declare_dram_parameter("h0mB", [128, 16 * HID], BF16, isOutput=False)
    w2f_d = nc.declare_dram_parameter("w2f", [128, 2 * HID2], BF16, isOutput=False)
    out_ds = [
        nc.declare_dram_parameter(f"o{b:02d}", [1, _jw(b)], F32, isOutput=True)
        for b in range(NB)
    ]
    boff = np.cumsum([0] + [N + W + 2 * _fw(b) for b in range(NB)])

    NI = NB + 3  # PE pipeline iterations (skew 3)

    # ---- DVE op-order counter: ops per DVE iter i are sc(i-4), h2c(i-1),
    # h3c(i-2); a single semaphore counts them so consumers wait once. ----
    cnt = 0
    c_sc = {}
    c_h2c = {}
    c_h3c = {}
    for i in range(NB + 4):
        if 0 <= i - 4 < NB:
            cnt += 1
            c_sc[i - 4] = cnt
        if 0 <= i - 1 < NB:
            cnt += 1
            c_h2c[i - 1] = cnt
        if 0 <= i - 2 < NB:
            cnt += 1
            c_h3c[i - 2] = cnt

    ctx = ExitStack()
    with ctx:
        # ---- persistent SBUF ----
        h0fA = ctx.enter_context(nc.sbuf_tensor("h0fA_s", [128, 2, HID], FP8))
        h0fB = ctx.enter_context(nc.sbuf_tensor("h0fB_s", [128, HID], BF16))
        h0mA = ctx.enter_context(
            nc.sbuf_tensor("h0mA_s", [128, 2, 32 * HID], FP8)
        )
        h0mB = ctx.enter_context(nc.sbuf_tensor("h0mB_s", [128, 16 * HID], BF16))
        w2f = ctx.enter_context(nc.sbuf_tensor("w2f_s", [128, 2 * HID2], BF16))
        pts = [
            ctx.enter_context(
                nc.sbuf_tensor(f"ptsb{s}", [128, N + W + 2 * FWMAX], BF16)
            )
            for s in range(PTBUF)
        ]
        at = [
            ctx.enter_context(nc.sbuf_tensor(f"atb{s}", [128, 2 * N], BF16))
            for s in range(3)
        ]
        h2sb = [
            ctx.enter_context(nc.sbuf_tensor(f"h2b{s}", [128, N], BF16))
            for s in range(3)
        ]
        h3sb = [
            ctx.enter_context(nc.sbuf_tensor(f"h3b{s}", [128, W], BF16))
            for s in range(3)
        ]
        srow = [
            ctx.enter_context(nc.sbuf_tensor(f"srowb{s}", [1, N], F32))
            for s in range(SRBUF)
        ]
        # ---- PSUM: 8 banks exactly ----
        aps = [
            [
                ctx.enter_context(nc.psum_tensor(f"apsb{p}{h}", [128, N], F32))
                for h in range(2)
            ]
            for p in range(2)
        ]  # aps[ctx%2][hc]
        h2ps = [
            ctx.enter_context(nc.psum_tensor(f"h2psb{s}", [128, N], F32))
            for s in range(2)
        ]
        h3ps = [
            ctx.enter_context(nc.psum_tensor(f"h3psb{s}", [128, W], F32))
            for s in range(2)
        ]

        # ---- semaphores ----
        sem_const = ctx.enter_context(nc.semaphore("sem_const"))
        sem_h0fA = ctx.enter_context(nc.semaphore("sem_h0fA"))
        sem_h0m = [
            ctx.enter_context(nc.semaphore(f"sem_h0m{p}"))
            for p in range(N_H0M_PIECES)
        ]
        sem_w2 = ctx.enter_context(nc.semaphore("sem_w2"))
        sem_pt = [
            ctx.enter_context(nc.semaphore(f"sem_pt{s}")) for s in range(PTBUF)
        ]
        sem_out = [
            ctx.enter_context(nc.semaphore(f"sem_out{s}")) for s in range(SRBUF)
        ]
        sem_warm = ctx.enter_context(nc.semaphore("sem_warm"))
        sem_mm1 = ctx.enter_context(nc.semaphore("sem_mm1"))
        sem_relu = ctx.enter_context(nc.semaphore("sem_relu"))
        sem_mm2 = ctx.enter_context(nc.semaphore("sem_mm2"))
        sem_mm3 = ctx.enter_context(nc.semaphore("sem_mm3"))
        sem_mm4 = ctx.enter_context(nc.semaphore("sem_mm4"))
        sem_dve = ctx.enter_context(nc.semaphore("sem_dve"))

        block = ctx.enter_context(nc.Block())

        def _load_pt(eng, p):
            s = p % PTBUF
            wid = N + W + 2 * _fw(p)
            eng.dma_start(
                pts[s][:, 0:wid], pts_d[:, boff[p] : boff[p] + wid]
            ).then_inc(sem_pt[s], 16)

        @block.sync
        def _(sync):
            # h0m piece 0 first: it gates the very first matmul (pt slots 0-4
            # load from the gpsimd queue, which starts its first DMA faster).
            # Ring slots 5-7 load from here so the fill runs on two
            # descriptor-generation pipelines (~1us per DMA each); a ring
            # slot's semaphore must always be fed from the same DMA queue.
            # The bulk h0m pieces go out on the ACT/DVE queues (idle at
            # startup) so they don't delay this queue's pt ring slots.
            sync.dma_start(h0mA[:, :, 0:HID], h0mA_d[:, :, 0:HID]).then_inc(
                sem_h0m[0], 16
            )
            sync.dma_start(h0fB[:, :], h0fB_d[:, :]).then_inc(sem_const, 16)
            sync.dma_start(
                h0mA[:, :, HID : 8 * HID], h0mA_d[:, :, HID : 8 * HID]
            ).then_inc(sem_h0m[1], 16)
            sync.dma_start(w2f[:, :], w2f_d[:, :]).then_inc(sem_w2, 16)
            # don't let the bulk ring-fill DMAs share bandwidth with pt
            # slots 0-1 (they gate iterations 0-1; slots 5-7 aren't needed
            # until ~10 iterations later)
            sync.wait_ge(sem_pt[1], 16)
            _load_pt(sync, 5)
            _load_pt(sync, 6)
            _load_pt(sync, 7)
            sync.dma_start(
                h0mA[:, :, 8 * HID : 16 * HID],
                h0mA_d[:, :, 8 * HID : 16 * HID],
            ).then_inc(sem_h0m[2], 16)
            sync.dma_start(h0fA[:, :, :], h0fA_d[:, :, :]).then_inc(sem_h0fA, 16)
            for i in range(NI):
                k = i - 3
                if 0 <= k < NB:
                    sync.wait_ge(sem_dve, c_sc[k])
                    sync.dma_start(
                        out_ds[k][:, :], srow[k % SRBUF][:, 0 : _jw(k)]
                    ).then_inc(sem_out[k % SRBUF], 16)
                p = i + PTBUF
                if p < NB and p % PTBUF >= 5:
                    sync.wait_ge(sem_mm3, i + 1)
                    _load_pt(sync, p)

        @block.gpsimd
        def _(g):
            # tiny memset so the ACT-table-preload activation has an
            # initialized operand (sim requirement)
            nc.gpsimd.memset(srow[SRBUF - 1][:, 0:8], 0.0).then_inc(sem_warm, 1)
            for p in range(0, min(5, NB)):
                if p >= 3:
                    # keep 3 slots' prefetch DMAs in flight (the bulk h0m /
                    # late-ring transfers are deferred until slots 0-2 land,
                    # so 3-way sharing is safe and fills the ring faster)
                    g.wait_ge(sem_pt[p - 3], 16)
                _load_pt(g, p)
            for i in range(NI):
                p = i + PTBUF
                if p < NB and p % PTBUF < 5:
                    g.wait_ge(sem_mm3, i + 1)
                    _load_pt(g, p)

        @block.tensor
        def _(te):
            te.wait_ge(sem_const, 16)
            for i in range(NI):
                if i == 1:
                    te.wait_ge(sem_w2, 16)
                if i == 32:
                    te.wait_ge(sem_h0fA, 16)  # shared DR pair for slots 32+
                # ---- mm1(i): DR fp8 (chunks 0,1) + bf16 chunk 2 ----
                if i < NB:
                    # aps[i%2] reuse (relu(i-2) drained) is implied by the
                    # previous iteration's sem_relu wait before mm2(i-2).
                    if i == 0 or _h0m_piece(i) != _h0m_piece(i - 1):
                        te.wait_ge(sem_h0m[_h0m_piece(i)], 16)
                    te.wait_ge(sem_pt[i % PTBUF], 16 * (i // PTBUF + 1))
                    movA = (
                        pts[i % PTBUF][:, 0:N]
                        .bitcast(FP8)
                        .rearrange("p (i n) -> p i n", i=2)
                    )
                    # both DR matmuls first, then both bf16 tails: each
                    # weight load hides under the previous matmul's stream
                    for hc in range(2):
                        if i < 32:
                            statA = h0mA[
                                :, :, i * HID + hc * 128 : i * HID + hc * 128 + 128
                            ]
                        else:
                            statA = h0fA[:, :, hc * 128 : hc * 128 + 128]
                        nc.tensor.matmul(
                            aps[i % 2][hc][:, :],
                            statA,
                            movA,
                            start=True,
                            stop=False,
                            perf_mode=DR,
                            skip_group_check=True,
                        )
                    for hc in range(2):
                        if i < 32:
                            statB = h0fB[:, hc * 128 : hc * 128 + 128]
                        else:
                            statB = h0mB[
                                :,
                                (i - 32) * HID + hc * 128 : (i - 32) * HID
                                + hc * 128
                                + 128,
                            ]
                        mm = nc.tensor.matmul(
                            aps[i % 2][hc][:, :],
                            statB,
                            pts[i % PTBUF][:, N + 1 : N + 1 + N],
                            start=False,
                            stop=True,
                            skip_group_check=True,
                        )
                        mm.then_inc(sem_mm1, 1)  # hc group done -> relu hc
                # ---- mm2(i-1): h2 = A@W2, bf16 F=128 x6 ----
                k = i - 1
                if 0 <= k < NB:
                    # h2ps[k%2] reuse (DVE h2c(k-2) drained) is implied by the
                    # previous iteration's sem_dve wait before mm3(k-1).
                    te.wait_ge(sem_relu, 2 * k + 2)
                    dst = h2ps[k % 2]
                    for jc in range(3):
                        for ht in range(2):
                            mm = nc.tensor.matmul(
                                dst[:, jc * 128 : (jc + 1) * 128],
                                at[k % 3][
                                    :, ht * N + jc * 128 : ht * N + jc * 128 + 128
                                ],
                                w2f[:, ht * HID2 : (ht + 1) * HID2],
                                start=(ht == 0),
                                stop=(ht == 1),
                            )
                    mm.then_inc(sem_mm2, 1)
                # ---- one sem_dve wait covers mm4(i-3) and mm3(i-2): h2c(k3),
                # the S-row drains of both aliased h3ps row 0s, h3c(k4), and
                # h2/h3 buffer reuse (all earlier in the DVE op order). ----
                k3 = i - 2
                k4 = i - 3
                if 0 <= k3 < NB:
                    te.wait_ge(sem_dve, c_sc[k3 - 2] if k3 >= 2 else c_h2c[k3])
                if 0 <= k4 < NB and (k4 == 0 or k4 == NB - 1):
                    # k4=0: mm3(1)'s c_h2c[1] wait precedes h3c(0) in the DVE
                    # order; k4=NB-1: no mm3 in this iteration.
                    te.wait_ge(sem_dve, c_h3c[k4])
                # ---- mm4(i-3): S row into h3ps[k4%2] partition 0, F=8k+8;
                # runs before mm3 so its sem_mm4 inc releases the next DVE
                # iteration's S-row drain earlier. ----
                if 0 <= k4 < NB:
                    mm = nc.tensor.matmul(
                        h3ps[k4 % 2][0:1, 0 : _jw(k4)],
                        h3sb[k4 % 3][:, 0:1],
                        h3sb[k4 % 3][:, 1 : 1 + _jw(k4)],
                        start=True,
                        stop=True,
                    )
                    mm.then_inc(sem_mm4, 1)
                # ---- mm3(i-2): h3T truncated (col 0 = d), bf16 F=8k+10 x3 ----
                k = k3
                if 0 <= k < NB:
                    fw = _fw(k)
                    dst = h3ps[k % 2]
                    for t in range(3):
                        if t < 2:
                            mov = pts[k % PTBUF][
                                :, N + W + t * fw : N + W + (t + 1) * fw
                            ]
                        else:
                            mov = pts[k % PTBUF][:, N : N + fw]
                        mm = nc.tensor.matmul(
                            dst[:, 0:fw],
                            h2sb[k % 3][:, t * 128 : (t + 1) * 128],
                            mov,
                            start=(t == 0),
                            stop=(t == 2),
                        )
                    mm.then_inc(sem_mm3, 1)

        @block.scalar
        def _(sc):
            # dummy activation: forces the ~1.3us ACT_TABLE_LOAD now, off
            # the first relu's critical path
            sc.wait_ge(sem_warm, 1)
            nc.scalar.activation(
                srow[SRBUF - 1][0:1, 0:8], srow[SRBUF - 1][0:1, 0:8], AFT.Relu
            ).then_inc(sem_warm, 1)
            # bulk h0m transfers wait until pt slot 4 lands: the whole ring
            # fill (slots 0-7, needed by iterations 0-7) gets DMA bandwidth
            # priority; these pieces aren't needed until iteration 16+
            sc.wait_ge(sem_pt[4], 16)
            for p in (3, 4):  # h0mA pieces 3,4 (slots 16-23, 24-31)
                sc.dma_start(
                    h0mA[:, :, 8 * (p - 1) * HID : 8 * p * HID],
                    h0mA_d[:, :, 8 * (p - 1) * HID : 8 * p * HID],
                ).then_inc(sem_h0m[p], 16)
            for p in range(2):  # h0mB pieces 5,6 (slots 32-39, 40-47)
                sc.dma_start(
                    h0mB[:, 8 * p * HID : 8 * (p + 1) * HID],
                    h0mB_d[:, 8 * p * HID : 8 * (p + 1) * HID],
                ).then_inc(sem_h0m[p + 5], 16)
            for i in range(NI):
                k = i
                if k < NB:
                    if k >= 3:
                        sc.wait_ge(sem_mm2, k - 2)  # at[k%3] reuse
                    for hc in range(2):
                        sc.wait_ge(sem_mm1, 2 * k + hc + 1)
                        nc.scalar.activation(
                            at[k % 3][:, hc * N : (hc + 1) * N],
                            aps[k % 2][hc][:, :],
                            AFT.Relu,
                        ).then_inc(sem_relu, 1)

        @block.vector
        def _(ve):
            # order the ACT-table dummy's srow access before this engine's
            # S-row drains (no other sync path connects them)
            ve.wait_ge(sem_warm, 2)
            for i in range(NB + 4):
                k = i - 4
                if 0 <= k < NB:
                    # S-row drain first: frees h3ps[k%2] row 0 for mm3(k+2)
                    # in the same PE iteration.
                    ve.wait_ge(sem_mm4, k + 1)
                    if k >= SRBUF:
                        ve.wait_ge(sem_out[k % SRBUF], 16 * (k // SRBUF))
                    nc.vector.tensor_copy(
                        srow[k % SRBUF][:, 0 : _jw(k)],
                        h3ps[k % 2][0:1, 0 : _jw(k)],
                    ).then_inc(sem_dve, 1)
                k = i - 1
                if 0 <= k < NB:
                    # h2sb[k%3] reuse (mm3(k-3) done) is implied by the
                    # previous iteration's sem_mm3 wait before h3c(k-1).
                    ve.wait_ge(sem_mm2, k + 1)
                    nc.vector.tensor_copy(
                        h2sb[k % 3][:, :], h2ps[k % 2][:, :]
                    ).then_inc(sem_dve, 1)
                k = i - 2
                if 0 <= k < NB:
                    if k >= 3:
                        ve.wait_ge(sem_mm4, k - 2)  # h3sb[k%3] reuse
                    ve.wait_ge(sem_mm3, k + 1)
                    nc.vector.tensor_copy(
                        h3sb[k % 3][:, 0 : _fw(k)],
                        h3ps[k % 2][:, 0 : _fw(k)],
                    ).then_inc(sem_dve, 1)

    return nc


def _get_nc() -> bass.Bass:
    if "nc" not in _NC_CACHE:
        _NC_CACHE["nc"] = _build_nc()
    return _NC_CACHE["nc"]


def _host_inputs(z, x, partials, W1, W2):
    """Per-core input dicts (list of NCORES)."""
    H0 = z[0] @ W1[:D]  # [384, 256] f32
    w1r = W1[D]  # [256]
    w2f = (
        np.ascontiguousarray(W2.reshape(2, 128, HID2).transpose(1, 0, 2))
        .reshape(128, 2 * HID2)
        .astype(BF)
    )
    # shared stationaries: chunks 0,1 as fp8 DR pair, chunk 2 bf16
    h0fA = np.ascontiguousarray(
        H0[: 2 * 128].reshape(2, 128, HID).transpose(1, 0, 2)
    ).astype(F8)  # [128, 2, 256]
    h0fB = np.ascontiguousarray(H0[2 * 128 :]).astype(BF)  # [128, 256]

    ptT = np.ascontiguousarray(partials.transpose(0, 2, 1))  # ptT[g,j,i]=P_g[i,j]
    ar = np.arange(N)
    prow = partials[ar, ar, :]  # [384, 384]  P_g[g, :]

    boff = np.cumsum([0] + [N + W + 2 * _fw(b) for b in range(NB)])
    btot = int(boff[-1])

    in_maps = []
    for c in range(NCORES):
        gs = np.arange(NB) * NCORES + c  # slot b -> context g = 8b + c
        ptTc = ptT[gs]  # [NB, 384, 384] (j, i) per slot
        # fp8 DR pair, chunks 0,1: [NB, 128, 2, N] -> bytes as bf16 cols
        ptA = np.ascontiguousarray(
            ptTc[:, : 2 * 128, :].reshape(NB, 2, 128, N).transpose(0, 2, 1, 3)
        ).astype(F8)
        ptAv = ptA.reshape(NB, 128, 2 * N).view(BF)  # [NB, 128, N]
        # pts: one bf16 blob per slot (see _build_nc for the layout)
        pts = np.zeros((128, btot), dtype=BF)
        for b in range(NB):
            fw = _fw(b)
            jw = min(fw - 1, N)  # slot 47's fw-1 = 385 > N: last col stays 0
            o = int(boff[b])
            pts[:, o : o + N] = ptAv[b]
            pts[:, o + N] = prow[gs[b]][2 * 128 :].astype(BF)
            pts[:, o + N + 1 : o + N + 1 + N] = ptTc[b, 2 * 128 :, :].astype(BF)
            for t in range(2):
                ot = o + N + W + t * fw
                pts[:, ot] = prow[gs[b]][t * 128 : (t + 1) * 128].astype(BF)
                pts[:, ot + 1 : ot + 1 + jw] = ptTc[
                    b, t * 128 : (t + 1) * 128, 0:jw
                ].astype(BF)
        # per-slot modified stationaries (row g += W1r)
        h0mA = np.empty((NB // 3 * 2, 128, 2, HID), dtype=F8)  # slots 0..31
        for b in range(32):
            g = 8 * b + c
            pair = H0[: 2 * 128].reshape(2, 128, HID).copy()
            pair[g // 128, g % 128] = H0[g] + w1r
            h0mA[b] = pair.transpose(1, 0, 2).astype(F8)
        h0mA = np.ascontiguousarray(h0mA.transpose(1, 2, 0, 3)).reshape(
            128, 2, 32 * HID
        )
        h0mB = np.empty((16, 128, HID), dtype=np.float32)  # slots 32..47
        for b in range(32, NB):
            g = 8 * b + c
            chunk = H0[2 * 128 :].copy()
            chunk[g - 2 * 128] = H0[g] + w1r
            h0mB[b - 32] = chunk
        h0mB = (
            np.ascontiguousarray(h0mB.transpose(1, 0, 2))
            .reshape(128, 16 * HID)
            .astype(BF)
        )
        in_maps.append(
            {
                "pts": pts,
                "h0fA": h0fA,
                "h0fB": h0fB,
                "h0mA": h0mA,
                "h0mB": h0mB,
                "w2f": w2f,
            }
        )
    return in_maps


def kernel(z, x, partials, W1, W2):
    global LAST_RESULT
    z = np.asarray(z, dtype=np.float32)
    x = np.asarray(x, dtype=np.float32)
    partials = np.asarray(partials, dtype=np.float32)
    W1 = np.asarray(W1, dtype=np.float32)
    W2 = np.asarray(W2, dtype=np.float32)

    in_maps = _host_inputs(z, x, partials, W1, W2)
    nc = _get_nc()
    res = run_bass_kernel_spmd(
        nc,
        in_maps,
        core_ids=list(range(NCORES)),
        trace=os.environ.get("KERNEL_TRACE", "0") not in ("0", ""),
    )
    LAST_RESULT = res
    S = np.zeros((N, N), dtype=np.float32)
    for c in range(NCORES):
        for b in range(NB):
            S[8 * b + c, 0 : _jw(b)] = np.asarray(
                res.results[c][f"o{b:02d}"], np.float32
            )[0]
    sup = np.tril(S)
    sup = (sup + sup.T) * np.float32(0.5)
    return (x + sup).astype(np.float32)
